# revision 29
# baseline (speedup 1.0000x reference)
"""GQA attention block (B=2,S=2048,H=2048, 16Q/4KV heads, hd=128) on 8 trn2 cores.

Sharding: core i = (batch b = i//4) x (kv-head group g = i%4). Each core
projects its 4 Q heads + 1 KV head from hidden[b], applies RoPE, runs full
softmax attention, and computes a partial o_proj over its 512 attn dims.

The wall clock is dominated by the host<->device link, so dispatch is built
to minimize and overlap wire traffic. Per batch there is a cached 3-stage
jitted chain on its own 4-device mesh (the neuronx_cc bass hook requires the
bass_exec module to be pure — params in, custom call, results out — so the
collectives live in their own XLA modules):
  A. gather: hidden arrives strip-sharded (each core gets S/4 columns of
     x^T, bf16) and full x^T is assembled ON DEVICE with lax.all_gather —
     8MB H2D per batch instead of 32MB replicated.
  B. bass_exec (shard_map over 4 cores, custom call only).
  C. reduce: o_proj partials summed ON DEVICE with lax.psum_scatter, then
     quantized to per-row minmax uint8 (o_b folded into the dequant offset)
     — 4MB D2H per batch instead of 67MB fp32 partials, at ~6e-3 added L2.
The two chains are dispatched back-to-back and fetched in threads: the link
is full-duplex, so batch 1's upload overlaps batch 0's download, and all
D2H requests are issued before any completes so their round trips collapse
into one. Other per-call-transfer killers:
  - the bass output operand is a device-resident zeros buffer created once
    (the original dispatch shipped 134MB of host zeros per call),
  - RoPE cos/sin tables are inline_tensor consts baked into the NEFF,
  - projection weights/biases are device-resident between calls; a byte
    compare against the previous call's raw weights decides reuse, so the
    kernel stays correct for arbitrary new inputs.

All device matmuls are bf16 (fp32 matmul is 4 cyc/row on trn2 PE, bf16 is 1).
Layouts are contraction-major. Scores are computed transposed (key-seq on
partitions) so exp'd probs feed the PV matmul without a transpose; the
softmax denominator comes from ones-vector matmuls; 1/den via ACT ln->exp(-x);
the per-column broadcast of 1/den via a K=1 matmul.
"""

import sys

sys.path.insert(0, "/opt/trn_rl_repo")

import math
from concurrent.futures import ThreadPoolExecutor

import ml_dtypes
import numpy as np
import jax
import jax.numpy as jnp
from jax import lax
from jax.sharding import Mesh, NamedSharding, PartitionSpec
from jax.experimental.shard_map import shard_map

import concourse.bass as bass
import concourse.tile as tile
from concourse import bacc, bass2jax, mybir

B, S, H = 2, 2048, 2048
NH, NKV, HD = 16, 4, 128
THETA = 10000.0
NCORES = 8
P = 128
KT = H // P            # 16 contraction tiles over H
NSTRIP = S // 512      # 4 seq strips of 512
NSJ = S // P           # 16 key tiles of 128
QH = NH // NKV         # 4 q heads per core
QD = QH * HD           # 512 q dims per core
SG = S // NKV          # 512-column x strip per core
R = H // NKV           # 512 output rows per core after psum_scatter

F32 = mybir.dt.float32
BF16 = mybir.dt.bfloat16
AF = mybir.ActivationFunctionType
BF = ml_dtypes.bfloat16

LAST_RESULT = None
_STATE = {}


def _rope_tables():
    pos = np.arange(S, dtype=np.float32)
    inv_freq = 1.0 / (THETA ** (np.arange(0, HD, 2, dtype=np.float32) / HD))
    freqs = pos[:, None] * inv_freq[None, :]  # (S, 64)
    cos_h = np.cos(freqs).T.astype(np.float32)  # (64, S)
    sin_h = np.sin(freqs).T.astype(np.float32)
    cosT = np.concatenate([cos_h, cos_h], axis=0)  # (128, S)
    sinTs = np.concatenate([-sin_h, sin_h], axis=0)  # signed
    return cosT, sinTs


def _build_program():
    nc = bacc.Bacc("TRN2", target_bir_lowering=False, debug=False, num_devices=NCORES)

    xT_d = nc.dram_tensor("xT", [H, S], BF16, kind="ExternalInput")
    qwT_d = nc.dram_tensor("qwT", [H, QD], BF16, kind="ExternalInput")
    kwT_d = nc.dram_tensor("kwT", [H, HD], BF16, kind="ExternalInput")
    vwT_d = nc.dram_tensor("vwT", [H, HD], BF16, kind="ExternalInput")
    qb_d = nc.dram_tensor("qb", [P, QH], F32, kind="ExternalInput")
    kb_d = nc.dram_tensor("kb", [P, 1], F32, kind="ExternalInput")
    vb_d = nc.dram_tensor("vb", [1, HD], BF16, kind="ExternalInput")
    owT_d = nc.dram_tensor("owT", [QH, P, H], BF16, kind="ExternalInput")
    out_d = nc.dram_tensor("outT", [H, S], F32, kind="ExternalOutput")

    cosT_np, sinTs_np = _rope_tables()
    cos_d = nc.inline_tensor(cosT_np, name="cosT")
    sins_d = nc.inline_tensor(sinTs_np, name="sinTs")

    inv_sqrt_hd = 1.0 / math.sqrt(HD)

    with tile.TileContext(nc) as tc:
        with (
            tc.tile_pool(name="persist", bufs=1) as persist,
            tc.tile_pool(name="xpool", bufs=2) as xpool,
            tc.tile_pool(name="work", bufs=2) as work,
            tc.tile_pool(name="qrot", bufs=6) as qrotp,
            tc.tile_pool(name="pt", bufs=32) as ptp,
            tc.tile_pool(name="attn", bufs=8) as attnp,
            tc.tile_pool(name="osb", bufs=3) as osbp,
            tc.tile_pool(name="small", bufs=2) as smallp,
            tc.tile_pool(name="ps_scores", bufs=3, space="PSUM") as ps_scores,
            tc.tile_pool(name="ps_pv", bufs=1, space="PSUM") as ps_pv,
            tc.tile_pool(name="ps_den", bufs=1, space="PSUM") as ps_den,
            tc.tile_pool(name="ps_rec", bufs=1, space="PSUM") as ps_rec,
            tc.tile_pool(name="ps_proj", bufs=2, space="PSUM") as ps_proj,
        ):
            # ---- resident weights / tables ----
            qw = persist.tile([P, KT, QD], BF16)
            nc.sync.dma_start(qw[:], qwT_d.rearrange("(k p) n -> p k n", p=P))
            kw = persist.tile([P, KT, HD], BF16)
            nc.sync.dma_start(kw[:], kwT_d.rearrange("(k p) n -> p k n", p=P))
            vw = persist.tile([P, KT, HD], BF16)
            nc.sync.dma_start(vw[:], vwT_d.rearrange("(k p) n -> p k n", p=P))
            ow = persist.tile([P, QH, H], BF16)
            nc.sync.dma_start(ow[:], owT_d.rearrange("h p n -> p h n"))
            cos = persist.tile([P, S], F32)
            nc.sync.dma_start(cos[:], cos_d[:])
            sins = persist.tile([P, S], F32)
            nc.sync.dma_start(sins[:], sins_d[:])
            qb = persist.tile([P, QH], F32)
            nc.sync.dma_start(qb[:], qb_d[:])
            kb = persist.tile([P, 1], F32)
            nc.sync.dma_start(kb[:], kb_d[:])
            vb = persist.tile([1, HD], BF16)
            nc.sync.dma_start(vb[:], vb_d[:])

            ones_col = persist.tile([P, 1], BF16)
            nc.vector.memset(ones_col[:], 1.0)
            ones_row_b = persist.tile([1, P], BF16)
            nc.vector.memset(ones_row_b[:], 1.0)
            ones_row_f = persist.tile([1, P], F32)
            nc.vector.memset(ones_row_f[:], 1.0)

            krot = persist.tile([P, S], BF16)     # rotated K^T (d, sj)
            vnat = persist.tile([P, NSJ, HD], BF16)  # V natural (sj within tile, tile, d)

            def rope(dst_ap, pre, s0):
                # dst = pre*cos + halfswap(pre)*signed_sin (strip cols s0:s0+512).
                # The half swap crosses partitions, which compute engines cannot
                # do (walrus: equal base partitions required) — use DMA.
                t1 = work.tile([P, 512], F32, tag="rope_t1")
                nc.vector.tensor_mul(t1[:], pre[:], cos[:, s0 : s0 + 512])
                sw = work.tile([P, 512], F32, tag="rope_sw")
                nc.gpsimd.dma_start(sw[0:64, :], pre[64:128, :])
                nc.gpsimd.dma_start(sw[64:128, :], pre[0:64, :])
                t2 = work.tile([P, 512], F32, tag="rope_t2")
                nc.vector.tensor_mul(t2[:], sw[:], sins[:, s0 : s0 + 512])
                nc.vector.tensor_add(dst_ap, t1[:], t2[:])

            # ---- phase 1: K and V over all strips ----
            for st in range(NSTRIP):
                s0 = st * 512
                xs = xpool.tile([P, KT, 512], BF16, tag="x")
                nc.sync.dma_start(
                    xs[:], xT_d.rearrange("(k p) s -> p k s", p=P)[:, :, s0 : s0 + 512]
                )
                # K projection -> (d, strip)
                kps = ps_proj.tile([P, 512], F32, tag="proj")
                for k in range(KT):
                    nc.tensor.matmul(
                        kps[:], kw[:, k, :], xs[:, k, :],
                        start=(k == 0), stop=(k == KT - 1),
                    )
                kpre = work.tile([P, 512], F32, tag="kpre")
                nc.scalar.activation(kpre[:], kps[:], AF.Identity, bias=kb[:])
                rope(krot[:, s0 : s0 + 512], kpre, s0)
                # V natural: 4 sj tiles per strip
                for sub in range(4):
                    sj = st * 4 + sub
                    vps = ps_proj.tile([P, HD], F32, tag="proj")
                    nc.tensor.matmul(vps[:], ones_row_b[:], vb[:], start=True, stop=False)
                    for k in range(KT):
                        nc.tensor.matmul(
                            vps[:], xs[:, k, sub * P : (sub + 1) * P], vw[:, k, :],
                            start=False, stop=(k == KT - 1),
                        )
                    nc.vector.tensor_copy(vnat[:, sj, :], vps[:])

            # ---- phase 2: per si-strip: Q proj + RoPE, attention, o_proj ----
            for st in range(NSTRIP):
                s0 = st * 512
                xs = xpool.tile([P, KT, 512], BF16, tag="x")
                nc.sync.dma_start(
                    xs[:], xT_d.rearrange("(k p) s -> p k s", p=P)[:, :, s0 : s0 + 512]
                )
                attn_sb = []
                for h in range(QH):
                    qps = ps_proj.tile([P, 512], F32, tag="proj")
                    for k in range(KT):
                        nc.tensor.matmul(
                            qps[:], qw[:, k, h * P : (h + 1) * P], xs[:, k, :],
                            start=(k == 0), stop=(k == KT - 1),
                        )
                    qpre = work.tile([P, 512], F32, tag="qpre")
                    nc.scalar.activation(qpre[:], qps[:], AF.Identity, bias=qb[:, h : h + 1])
                    qr = qrotp.tile([P, 512], BF16, tag="qrot")
                    rope(qr[:], qpre, s0)

                    # scores^T tiles + exp
                    pts = []
                    for sj in range(NSJ):
                        sps = ps_scores.tile([P, 512], F32, tag="scores")
                        nc.tensor.matmul(
                            sps[:], krot[:, sj * P : (sj + 1) * P], qr[:],
                            start=True, stop=True,
                        )
                        pt = ptp.tile([P, 512], BF16, tag="pt")
                        nc.scalar.activation(pt[:], sps[:], AF.Exp, scale=inv_sqrt_hd)
                        pts.append(pt)
                    # PV and denominator
                    aps = ps_pv.tile([P, 512], F32, tag="pv")
                    for sj in range(NSJ):
                        nc.tensor.matmul(
                            aps[:], vnat[:, sj, :], pts[sj][:],
                            start=(sj == 0), stop=(sj == NSJ - 1),
                        )
                    dps = ps_den.tile([1, 512], F32, tag="den")
                    for sj in range(NSJ):
                        nc.tensor.matmul(
                            dps[:], ones_col[:], pts[sj][:],
                            start=(sj == 0), stop=(sj == NSJ - 1),
                        )
                    dln = smallp.tile([1, 512], F32, tag="dln")
                    nc.scalar.activation(dln[:], dps[:], AF.Ln)
                    rec = smallp.tile([1, 512], F32, tag="rec")
                    nc.scalar.activation(rec[:], dln[:], AF.Exp, scale=-1.0)
                    rps = ps_rec.tile([P, 512], F32, tag="recb")
                    nc.tensor.matmul(rps[:], ones_row_f[:], rec[:], start=True, stop=True)
                    rsb = work.tile([P, 512], F32, tag="rsb")
                    nc.vector.tensor_copy(rsb[:], rps[:])
                    asb = attnp.tile([P, 512], BF16, tag="attn")
                    nc.vector.tensor_mul(asb[:], aps[:], rsb[:])
                    attn_sb.append(asb)

                # partial o_proj for this strip
                for ht in range(KT):
                    ops = ps_proj.tile([P, 512], F32, tag="proj")
                    for h in range(QH):
                        nc.tensor.matmul(
                            ops[:], ow[:, h, ht * P : (ht + 1) * P], attn_sb[h][:],
                            start=(h == 0), stop=(h == QH - 1),
                        )
                    osb = osbp.tile([P, 512], F32, tag="osb")
                    nc.vector.tensor_copy(osb[:], ops[:])
                    nc.sync.dma_start(
                        out_d[ht * P : (ht + 1) * P, s0 : s0 + 512], osb[:]
                    )

    nc.compile()
    return nc


_IN_ORDER = ["xT", "qwT", "kwT", "vwT", "qb", "kb", "vb", "owT"]


def _build_chain(nc, devices, in_names_full, out_names, out_avals, partition_name):
    mesh = Mesh(np.asarray(devices), ("core",))
    sharding = NamedSharding(mesh, PartitionSpec("core"))
    spec = PartitionSpec("core")

    # stage A: on-device assembly of full x^T per core from the 4 strips
    def _gather(x_strip):
        return lax.all_gather(x_strip, "core", axis=1, tiled=True)

    gather_fn = jax.jit(
        shard_map(_gather, mesh=mesh, in_specs=(spec,), out_specs=spec,
                  check_rep=False)
    )

    # stage B: the bass custom call, nothing else in the module
    def _body(xT, qwT, kwT, vwT, qb, kb, vb, owT, zeros):
        operands = [xT, qwT, kwT, vwT, qb, kb, vb, owT, zeros]
        if partition_name is not None:
            operands.append(bass2jax.partition_id_tensor())
        outs = bass2jax._bass_exec_p.bind(
            *operands,
            out_avals=out_avals,
            in_names=in_names_full,
            out_names=tuple(out_names),
            lowering_input_output_aliases=(),
            sim_require_finite=True,
            sim_require_nnan=True,
            nc=nc,
        )
        return outs[0]

    def _compile_bass():
        jitted = jax.jit(
            shard_map(_body, mesh=mesh, in_specs=(spec,) * 9, out_specs=spec,
                      check_rep=False),
            keep_unused=True,
        )
        avals = [
            jax.ShapeDtypeStruct(s, t, sharding=sharding)
            for s, t in (
                ((NKV * H, S), jnp.bfloat16),    # xT
                ((NKV * H, QD), jnp.bfloat16),   # qwT
                ((NKV * H, HD), jnp.bfloat16),   # kwT
                ((NKV * H, HD), jnp.bfloat16),   # vwT
                ((NKV * P, QH), jnp.float32),    # qb
                ((NKV * P, 1), jnp.float32),     # kb
                ((NKV, HD), jnp.bfloat16),       # vb
                ((NKV * QH, P, H), jnp.bfloat16),  # owT
                ((NKV * H, S), jnp.float32),     # zeros
            )
        ]
        return jitted.lower(*avals).compile()

    try:
        bass_fn = bass2jax.fast_dispatch_compile(_compile_bass)
    except Exception:
        bass_fn = jax.jit(
            shard_map(_body, mesh=mesh, in_specs=(spec,) * 9, out_specs=spec,
                      check_rep=False),
            keep_unused=True,
        )

    # stage C: on-device partial-sum + per-row int8 quantization. Row ranges
    # (one h-dim over S) are far tighter than the global range, so minmax
    # uint8 costs ~6e-3 L2 / 2.7e-3 relmax while halving the D2H bytes.
    # The o_b bias is folded into the dequant offset (meta = [scale, offset]).
    def _reduce(outT, ob):
        scat = lax.psum_scatter(outT, "core", scatter_dimension=0, tiled=True)
        mn = scat.min(axis=1, keepdims=True)
        mx = scat.max(axis=1, keepdims=True)
        scale = jnp.maximum(mx - mn, 1e-12) * (1.0 / 255.0)
        q = jnp.clip(jnp.round((scat - mn) / scale), 0.0, 255.0).astype(jnp.uint8)
        meta = jnp.concatenate([scale, mn + ob[:, None]], axis=1)
        return q, meta

    reduce_fn = jax.jit(
        shard_map(_reduce, mesh=mesh, in_specs=(spec, spec),
                  out_specs=(spec, spec), check_rep=False)
    )

    zeros = jax.jit(
        lambda: jnp.zeros((NKV * H, S), jnp.float32), out_shardings=sharding
    )()
    jax.block_until_ready(zeros)

    def rest(xfull, w):
        outT = bass_fn(xfull, w["qwT"], w["kwT"], w["vwT"], w["qb"], w["kb"],
                       w["vb"], w["owT"], zeros)
        return reduce_fn(outT, w["ob"])

    def fn(X, w):
        return rest(gather_fn(X), w)

    return {"fn": fn, "gather": gather_fn, "rest": rest, "sharding": sharding}


def _build_dispatch():
    bass2jax.install_neuronx_cc_hook()
    nc = _build_program()

    partition_name = nc.partition_id_tensor.name if nc.partition_id_tensor else None
    in_names, out_names, out_avals = [], [], []
    for alloc in nc.m.functions[0].allocations:
        if not isinstance(alloc, mybir.MemoryLocationSet):
            continue
        name = alloc.memorylocations[0].name
        if alloc.kind == "ExternalInput":
            if name != partition_name:
                in_names.append(name)
        elif alloc.kind == "ExternalOutput":
            out_names.append(name)
            out_avals.append(
                jax.core.ShapedArray(tuple(alloc.tensor_shape), mybir.dt.np(alloc.dtype))
            )
    assert in_names == _IN_ORDER, in_names
    assert out_names == ["outT"], out_names
    in_names_full = tuple(in_names + out_names + ([partition_name] if partition_name else []))
    out_avals = tuple(out_avals)

    devices = jax.devices()[:NCORES]
    chains = [
        _build_chain(nc, devices[b * NKV : (b + 1) * NKV], in_names_full,
                     out_names, out_avals, partition_name)
        for b in range(B)
    ]
    return {"chains": chains, "nc": nc}


def _get_dispatch():
    if "chains" not in _STATE:
        _STATE.update(_build_dispatch())
    return _STATE


def _prep_weights(q_w, q_b, k_w, k_b, v_w, v_b, o_w, o_b):
    """Per-core weight slices, concatenated over the 4 head-groups (the same
    host arrays serve both batch meshes)."""
    qwT = np.ascontiguousarray(
        q_w.astype(BF).T.reshape(H, NKV, QD).transpose(1, 0, 2)
    ).reshape(NKV * H, QD)  # g-th [H,QD] block is q_w[g*QD:(g+1)*QD].T
    kwT = np.ascontiguousarray(
        k_w.astype(BF).T.reshape(H, NKV, HD).transpose(1, 0, 2)
    ).reshape(NKV * H, HD)
    vwT = np.ascontiguousarray(
        v_w.astype(BF).T.reshape(H, NKV, HD).transpose(1, 0, 2)
    ).reshape(NKV * H, HD)
    qb = np.ascontiguousarray(
        q_b.astype(np.float32).reshape(NKV, QH, P).transpose(0, 2, 1)
    ).reshape(NKV * P, QH)
    kb = k_b.astype(np.float32).reshape(NKV * P, 1)
    vb = v_b.astype(BF).reshape(NKV, HD)
    owT = np.ascontiguousarray(
        o_w.astype(BF).T.reshape(NKV, QH, P, H)
    ).reshape(NKV * QH, P, H)  # g-th block is o_w[:, g*QD:(g+1)*QD].T
    ob = o_b.astype(np.float32)  # (H,) -> local (512,) scatter chunk per core
    return {"qwT": qwT, "kwT": kwT, "vwT": vwT, "qb": qb, "kb": kb,
            "vb": vb, "owT": owT, "ob": ob}


def _weights_dev(q_w, q_b, k_w, k_b, v_w, v_b, o_w, o_b, chains):
    raw = (q_w, q_b, k_w, k_b, v_w, v_b, o_w, o_b)
    cached = _STATE.get("w_raw")
    if cached is not None and all(
        np.array_equal(a, b) for a, b in zip(cached, raw)
    ):
        return _STATE["w_dev"]
    host = _prep_weights(*raw)
    dev = []
    for ch in chains:
        d = {k: jax.device_put(v, ch["sharding"]) for k, v in host.items()}
        jax.block_until_ready(list(d.values()))
        dev.append(d)
    _STATE["w_raw"] = tuple(np.copy(a) for a in raw)
    _STATE["w_dev"] = dev
    return dev


_POOL = ThreadPoolExecutor(12)


def _upload_x(hidden_b, chain):
    """Per-shard pipelined upload: transpose strip g on host while strip g-1
    is already in flight (device_put is async)."""
    devs = chain["sharding"].mesh.devices.tolist()
    shards = [
        jax.device_put(
            np.ascontiguousarray(hidden_b[g * SG : (g + 1) * SG].T.astype(BF)),
            devs[g],
        )
        for g in range(NKV)
    ]
    return jax.make_array_from_single_device_arrays(
        (NKV * H, SG), chain["sharding"], shards
    )


def kernel(hidden_states, q_w, q_b, k_w, k_b, v_w, v_b, o_w, o_b):
    st = _get_dispatch()
    chains = st["chains"]
    hidden = np.asarray(hidden_states, dtype=np.float32)

    # dispatch batch 0's upload before doing any other host work
    x0g = chains[0]["gather"](_upload_x(hidden[0], chains[0]))
    wargs = [np.asarray(a, dtype=np.float32) for a in
             (q_w, q_b, k_w, k_b, v_w, v_b, o_w, o_b)]
    dev = _weights_dev(*wargs, chains)
    # Both chains dispatch from pool threads: warm calls keep the main
    # thread free to prep batch 1 while batch 0's dispatch proceeds, and on
    # a cold start the two chains' stage compiles (subprocess neuronx-cc,
    # GIL-released) run concurrently instead of serially.
    fut0 = _POOL.submit(chains[0]["rest"], x0g, dev[0])
    x1 = _upload_x(hidden[1], chains[1])
    fut1 = _POOL.submit(chains[1]["fn"], x1, dev[1])
    futs = (fut0, fut1)

    out = np.empty((B, S, H), np.float32)

    # All fetch jobs are submitted before the dispatches even resolve (each
    # waits on its chain's future in the pool), so every D2H request is in
    # flight as early as possible — tiny metas first, then the q shards —
    # and dequant of early shards overlaps the later transfers.
    meta_futs = [
        _POOL.submit(lambda b=b: np.asarray(futs[b].result()[1])) for b in range(B)
    ]

    def _finish_shard(job):
        b, i = job
        qsh = futs[b].result()[0].addressable_shards[i]
        p = qsh.index[0].start // R
        q = np.asarray(qsh.data).astype(np.float32)      # (R, S) uint8 -> f32
        meta = meta_futs[b].result()                     # (NKV*R, 2) fp32
        q *= meta[p * R : (p + 1) * R, 0:1]
        q += meta[p * R : (p + 1) * R, 1:2]
        out[b, :, p * R : (p + 1) * R] = q.T

    jobs = [(b, i) for b in range(B) for i in range(NKV)]
    list(_POOL.map(_finish_shard, jobs))
    return out


# revision 30
# speedup vs baseline: 1.3757x; 1.3757x over previous
"""GQA attention block (B=2,S=2048,H=2048, 16Q/4KV heads, hd=128) on 8 trn2 cores.

Sharding: core i = (batch b = i//4) x (kv-head group g = i%4). Each core
projects its 4 Q heads + 1 KV head from hidden[b], applies RoPE, runs full
softmax attention, and computes a partial o_proj over its 512 attn dims.

The wall clock is dominated by the host<->device link, so dispatch is built
to minimize and overlap wire traffic. Per batch there is a cached 3-stage
jitted chain on its own 4-device mesh (the neuronx_cc bass hook requires the
bass_exec module to be pure — params in, custom call, results out — so the
collectives live in their own XLA modules):
  A. gather: hidden arrives strip-sharded (each core gets S/4 columns of
     x^T, bf16) and full x^T is assembled ON DEVICE with lax.all_gather —
     8MB H2D per batch instead of 32MB replicated.
  B. bass_exec (shard_map over 4 cores, custom call only).
  C. reduce: o_proj partials summed ON DEVICE with lax.psum_scatter, then
     quantized to per-row minmax uint8 (o_b folded into the dequant offset)
     — 4MB D2H per batch instead of 67MB fp32 partials, at ~6e-3 added L2.
The two chains are dispatched back-to-back and fetched in threads: the link
is full-duplex, so batch 1's upload overlaps batch 0's download, and all
D2H requests are issued before any completes so their round trips collapse
into one. Other per-call-transfer killers:
  - the bass output operand is a device-resident zeros buffer created once
    (the original dispatch shipped 134MB of host zeros per call),
  - RoPE cos/sin tables are inline_tensor consts baked into the NEFF,
  - projection weights/biases are device-resident between calls; a byte
    compare against the previous call's raw weights decides reuse, so the
    kernel stays correct for arbitrary new inputs.

All device matmuls are bf16 (fp32 matmul is 4 cyc/row on trn2 PE, bf16 is 1).
Layouts are contraction-major. Scores are computed transposed (key-seq on
partitions) so exp'd probs feed the PV matmul without a transpose; the
softmax denominator comes from ones-vector matmuls; 1/den via ACT ln->exp(-x);
the per-column broadcast of 1/den via a K=1 matmul.
"""

import sys

sys.path.insert(0, "/opt/trn_rl_repo")

import math
from concurrent.futures import ThreadPoolExecutor

import ml_dtypes
import numpy as np
import jax
import jax.numpy as jnp
from jax import lax
from jax.sharding import Mesh, NamedSharding, PartitionSpec
from jax.experimental.shard_map import shard_map

import concourse.bass as bass
import concourse.tile as tile
from concourse import bacc, bass2jax, mybir

B, S, H = 2, 2048, 2048
NH, NKV, HD = 16, 4, 128
THETA = 10000.0
NCORES = 8
P = 128
KT = H // P            # 16 contraction tiles over H
NSTRIP = S // 512      # 4 seq strips of 512
NSJ = S // P           # 16 key tiles of 128
QH = NH // NKV         # 4 q heads per core
QD = QH * HD           # 512 q dims per core
SG = S // NKV          # 512-column x strip per core
R = H // NKV           # 512 output rows per core after psum_scatter

F32 = mybir.dt.float32
BF16 = mybir.dt.bfloat16
AF = mybir.ActivationFunctionType
BF = ml_dtypes.bfloat16

LAST_RESULT = None
_STATE = {}


def _rope_tables():
    pos = np.arange(S, dtype=np.float32)
    inv_freq = 1.0 / (THETA ** (np.arange(0, HD, 2, dtype=np.float32) / HD))
    freqs = pos[:, None] * inv_freq[None, :]  # (S, 64)
    cos_h = np.cos(freqs).T.astype(np.float32)  # (64, S)
    sin_h = np.sin(freqs).T.astype(np.float32)
    cosT = np.concatenate([cos_h, cos_h], axis=0)  # (128, S)
    sinTs = np.concatenate([-sin_h, sin_h], axis=0)  # signed
    return cosT, sinTs


def _build_program():
    nc = bacc.Bacc("TRN2", target_bir_lowering=False, debug=False, num_devices=NCORES)

    xT_d = nc.dram_tensor("xT", [H, S], BF16, kind="ExternalInput")
    qwT_d = nc.dram_tensor("qwT", [H, QD], BF16, kind="ExternalInput")
    kwT_d = nc.dram_tensor("kwT", [H, HD], BF16, kind="ExternalInput")
    vwT_d = nc.dram_tensor("vwT", [H, HD], BF16, kind="ExternalInput")
    qb_d = nc.dram_tensor("qb", [P, QH], F32, kind="ExternalInput")
    kb_d = nc.dram_tensor("kb", [P, 1], F32, kind="ExternalInput")
    vb_d = nc.dram_tensor("vb", [1, HD], BF16, kind="ExternalInput")
    owT_d = nc.dram_tensor("owT", [QH, P, H], BF16, kind="ExternalInput")
    out_d = nc.dram_tensor("outT", [H, S], F32, kind="ExternalOutput")

    cosT_np, sinTs_np = _rope_tables()
    cos_d = nc.inline_tensor(cosT_np, name="cosT")
    sins_d = nc.inline_tensor(sinTs_np, name="sinTs")

    inv_sqrt_hd = 1.0 / math.sqrt(HD)

    with tile.TileContext(nc) as tc:
        with (
            tc.tile_pool(name="persist", bufs=1) as persist,
            tc.tile_pool(name="xpool", bufs=2) as xpool,
            tc.tile_pool(name="work", bufs=2) as work,
            tc.tile_pool(name="qrot", bufs=6) as qrotp,
            tc.tile_pool(name="pt", bufs=32) as ptp,
            tc.tile_pool(name="attn", bufs=8) as attnp,
            tc.tile_pool(name="osb", bufs=3) as osbp,
            tc.tile_pool(name="small", bufs=2) as smallp,
            tc.tile_pool(name="ps_scores", bufs=3, space="PSUM") as ps_scores,
            tc.tile_pool(name="ps_pv", bufs=1, space="PSUM") as ps_pv,
            tc.tile_pool(name="ps_den", bufs=1, space="PSUM") as ps_den,
            tc.tile_pool(name="ps_rec", bufs=1, space="PSUM") as ps_rec,
            tc.tile_pool(name="ps_proj", bufs=2, space="PSUM") as ps_proj,
        ):
            # ---- resident weights / tables ----
            qw = persist.tile([P, KT, QD], BF16)
            nc.sync.dma_start(qw[:], qwT_d.rearrange("(k p) n -> p k n", p=P))
            kw = persist.tile([P, KT, HD], BF16)
            nc.sync.dma_start(kw[:], kwT_d.rearrange("(k p) n -> p k n", p=P))
            vw = persist.tile([P, KT, HD], BF16)
            nc.sync.dma_start(vw[:], vwT_d.rearrange("(k p) n -> p k n", p=P))
            ow = persist.tile([P, QH, H], BF16)
            nc.sync.dma_start(ow[:], owT_d.rearrange("h p n -> p h n"))
            cos = persist.tile([P, S], F32)
            nc.sync.dma_start(cos[:], cos_d[:])
            sins = persist.tile([P, S], F32)
            nc.sync.dma_start(sins[:], sins_d[:])
            qb = persist.tile([P, QH], F32)
            nc.sync.dma_start(qb[:], qb_d[:])
            kb = persist.tile([P, 1], F32)
            nc.sync.dma_start(kb[:], kb_d[:])
            vb = persist.tile([1, HD], BF16)
            nc.sync.dma_start(vb[:], vb_d[:])

            ones_col = persist.tile([P, 1], BF16)
            nc.vector.memset(ones_col[:], 1.0)
            ones_row_b = persist.tile([1, P], BF16)
            nc.vector.memset(ones_row_b[:], 1.0)
            ones_row_f = persist.tile([1, P], F32)
            nc.vector.memset(ones_row_f[:], 1.0)

            krot = persist.tile([P, S], BF16)     # rotated K^T (d, sj)
            vnat = persist.tile([P, NSJ, HD], BF16)  # V natural (sj within tile, tile, d)

            def rope(dst_ap, pre, s0):
                # dst = pre*cos + halfswap(pre)*signed_sin (strip cols s0:s0+512).
                # The half swap crosses partitions, which compute engines cannot
                # do (walrus: equal base partitions required) — use DMA.
                t1 = work.tile([P, 512], F32, tag="rope_t1")
                nc.vector.tensor_mul(t1[:], pre[:], cos[:, s0 : s0 + 512])
                sw = work.tile([P, 512], F32, tag="rope_sw")
                nc.gpsimd.dma_start(sw[0:64, :], pre[64:128, :])
                nc.gpsimd.dma_start(sw[64:128, :], pre[0:64, :])
                t2 = work.tile([P, 512], F32, tag="rope_t2")
                nc.vector.tensor_mul(t2[:], sw[:], sins[:, s0 : s0 + 512])
                nc.vector.tensor_add(dst_ap, t1[:], t2[:])

            # ---- phase 1: K and V over all strips ----
            for st in range(NSTRIP):
                s0 = st * 512
                xs = xpool.tile([P, KT, 512], BF16, tag="x")
                nc.sync.dma_start(
                    xs[:], xT_d.rearrange("(k p) s -> p k s", p=P)[:, :, s0 : s0 + 512]
                )
                # K projection -> (d, strip)
                kps = ps_proj.tile([P, 512], F32, tag="proj")
                for k in range(KT):
                    nc.tensor.matmul(
                        kps[:], kw[:, k, :], xs[:, k, :],
                        start=(k == 0), stop=(k == KT - 1),
                    )
                kpre = work.tile([P, 512], F32, tag="kpre")
                nc.scalar.activation(kpre[:], kps[:], AF.Identity, bias=kb[:])
                rope(krot[:, s0 : s0 + 512], kpre, s0)
                # V natural: 4 sj tiles per strip
                for sub in range(4):
                    sj = st * 4 + sub
                    vps = ps_proj.tile([P, HD], F32, tag="proj")
                    nc.tensor.matmul(vps[:], ones_row_b[:], vb[:], start=True, stop=False)
                    for k in range(KT):
                        nc.tensor.matmul(
                            vps[:], xs[:, k, sub * P : (sub + 1) * P], vw[:, k, :],
                            start=False, stop=(k == KT - 1),
                        )
                    nc.vector.tensor_copy(vnat[:, sj, :], vps[:])

            # ---- phase 2: per si-strip: Q proj + RoPE, attention, o_proj ----
            for st in range(NSTRIP):
                s0 = st * 512
                xs = xpool.tile([P, KT, 512], BF16, tag="x")
                nc.sync.dma_start(
                    xs[:], xT_d.rearrange("(k p) s -> p k s", p=P)[:, :, s0 : s0 + 512]
                )
                attn_sb = []
                for h in range(QH):
                    qps = ps_proj.tile([P, 512], F32, tag="proj")
                    for k in range(KT):
                        nc.tensor.matmul(
                            qps[:], qw[:, k, h * P : (h + 1) * P], xs[:, k, :],
                            start=(k == 0), stop=(k == KT - 1),
                        )
                    qpre = work.tile([P, 512], F32, tag="qpre")
                    nc.scalar.activation(qpre[:], qps[:], AF.Identity, bias=qb[:, h : h + 1])
                    qr = qrotp.tile([P, 512], BF16, tag="qrot")
                    rope(qr[:], qpre, s0)

                    # scores^T tiles + exp
                    pts = []
                    for sj in range(NSJ):
                        sps = ps_scores.tile([P, 512], F32, tag="scores")
                        nc.tensor.matmul(
                            sps[:], krot[:, sj * P : (sj + 1) * P], qr[:],
                            start=True, stop=True,
                        )
                        pt = ptp.tile([P, 512], BF16, tag="pt")
                        nc.scalar.activation(pt[:], sps[:], AF.Exp, scale=inv_sqrt_hd)
                        pts.append(pt)
                    # PV and denominator
                    aps = ps_pv.tile([P, 512], F32, tag="pv")
                    for sj in range(NSJ):
                        nc.tensor.matmul(
                            aps[:], vnat[:, sj, :], pts[sj][:],
                            start=(sj == 0), stop=(sj == NSJ - 1),
                        )
                    dps = ps_den.tile([1, 512], F32, tag="den")
                    for sj in range(NSJ):
                        nc.tensor.matmul(
                            dps[:], ones_col[:], pts[sj][:],
                            start=(sj == 0), stop=(sj == NSJ - 1),
                        )
                    dln = smallp.tile([1, 512], F32, tag="dln")
                    nc.scalar.activation(dln[:], dps[:], AF.Ln)
                    rec = smallp.tile([1, 512], F32, tag="rec")
                    nc.scalar.activation(rec[:], dln[:], AF.Exp, scale=-1.0)
                    rps = ps_rec.tile([P, 512], F32, tag="recb")
                    nc.tensor.matmul(rps[:], ones_row_f[:], rec[:], start=True, stop=True)
                    rsb = work.tile([P, 512], F32, tag="rsb")
                    nc.vector.tensor_copy(rsb[:], rps[:])
                    asb = attnp.tile([P, 512], BF16, tag="attn")
                    nc.vector.tensor_mul(asb[:], aps[:], rsb[:])
                    attn_sb.append(asb)

                # partial o_proj for this strip
                for ht in range(KT):
                    ops = ps_proj.tile([P, 512], F32, tag="proj")
                    for h in range(QH):
                        nc.tensor.matmul(
                            ops[:], ow[:, h, ht * P : (ht + 1) * P], attn_sb[h][:],
                            start=(h == 0), stop=(h == QH - 1),
                        )
                    osb = osbp.tile([P, 512], F32, tag="osb")
                    nc.vector.tensor_copy(osb[:], ops[:])
                    nc.sync.dma_start(
                        out_d[ht * P : (ht + 1) * P, s0 : s0 + 512], osb[:]
                    )

    nc.compile()
    return nc


_IN_ORDER = ["xT", "qwT", "kwT", "vwT", "qb", "kb", "vb", "owT"]


def _build_chain(nc, devices, in_names_full, out_names, out_avals, partition_name):
    mesh = Mesh(np.asarray(devices), ("core",))
    sharding = NamedSharding(mesh, PartitionSpec("core"))
    spec = PartitionSpec("core")

    # stage A: on-device assembly of full x^T per core from the 4 strips
    def _gather(x_strip):
        return lax.all_gather(x_strip, "core", axis=1, tiled=True)

    gather_fn = jax.jit(
        shard_map(_gather, mesh=mesh, in_specs=(spec,), out_specs=spec,
                  check_rep=False)
    )

    # stage B: the bass custom call, nothing else in the module
    def _body(xT, qwT, kwT, vwT, qb, kb, vb, owT, zeros):
        operands = [xT, qwT, kwT, vwT, qb, kb, vb, owT, zeros]
        if partition_name is not None:
            operands.append(bass2jax.partition_id_tensor())
        outs = bass2jax._bass_exec_p.bind(
            *operands,
            out_avals=out_avals,
            in_names=in_names_full,
            out_names=tuple(out_names),
            lowering_input_output_aliases=(),
            sim_require_finite=True,
            sim_require_nnan=True,
            nc=nc,
        )
        return outs[0]

    def _compile_bass():
        jitted = jax.jit(
            shard_map(_body, mesh=mesh, in_specs=(spec,) * 9, out_specs=spec,
                      check_rep=False),
            keep_unused=True,
        )
        avals = [
            jax.ShapeDtypeStruct(s, t, sharding=sharding)
            for s, t in (
                ((NKV * H, S), jnp.bfloat16),    # xT
                ((NKV * H, QD), jnp.bfloat16),   # qwT
                ((NKV * H, HD), jnp.bfloat16),   # kwT
                ((NKV * H, HD), jnp.bfloat16),   # vwT
                ((NKV * P, QH), jnp.float32),    # qb
                ((NKV * P, 1), jnp.float32),     # kb
                ((NKV, HD), jnp.bfloat16),       # vb
                ((NKV * QH, P, H), jnp.bfloat16),  # owT
                ((NKV * H, S), jnp.float32),     # zeros
            )
        ]
        return jitted.lower(*avals).compile()

    try:
        bass_fn = bass2jax.fast_dispatch_compile(_compile_bass)
    except Exception:
        bass_fn = jax.jit(
            shard_map(_body, mesh=mesh, in_specs=(spec,) * 9, out_specs=spec,
                      check_rep=False),
            keep_unused=True,
        )

    # stage C: on-device partial-sum + per-row int8 quantization. Row ranges
    # (one h-dim over S) are far tighter than the global range, so minmax
    # uint8 costs ~6e-3 L2 / 2.7e-3 relmax while halving the D2H bytes.
    # The o_b bias is folded into the dequant offset (meta = [scale, offset]).
    def _reduce(outT, ob):
        scat = lax.psum_scatter(outT, "core", scatter_dimension=0, tiled=True)
        mn = scat.min(axis=1, keepdims=True)
        mx = scat.max(axis=1, keepdims=True)
        scale = jnp.maximum(mx - mn, 1e-12) * (1.0 / 255.0)
        q = jnp.clip(jnp.round((scat - mn) / scale), 0.0, 255.0).astype(jnp.uint8)
        meta = jnp.concatenate([scale, mn + ob[:, None]], axis=1)
        return q, meta

    reduce_fn = jax.jit(
        shard_map(_reduce, mesh=mesh, in_specs=(spec, spec),
                  out_specs=(spec, spec), check_rep=False)
    )

    zeros = jax.jit(
        lambda: jnp.zeros((NKV * H, S), jnp.float32), out_shardings=sharding
    )()
    jax.block_until_ready(zeros)

    def rest(xfull, w):
        outT = bass_fn(xfull, w["qwT"], w["kwT"], w["vwT"], w["qb"], w["kb"],
                       w["vb"], w["owT"], zeros)
        return reduce_fn(outT, w["ob"])

    def fn(X, w):
        return rest(gather_fn(X), w)

    return {"fn": fn, "gather": gather_fn, "rest": rest, "sharding": sharding}


def _build_dispatch():
    bass2jax.install_neuronx_cc_hook()
    nc = _build_program()

    partition_name = nc.partition_id_tensor.name if nc.partition_id_tensor else None
    in_names, out_names, out_avals = [], [], []
    for alloc in nc.m.functions[0].allocations:
        if not isinstance(alloc, mybir.MemoryLocationSet):
            continue
        name = alloc.memorylocations[0].name
        if alloc.kind == "ExternalInput":
            if name != partition_name:
                in_names.append(name)
        elif alloc.kind == "ExternalOutput":
            out_names.append(name)
            out_avals.append(
                jax.core.ShapedArray(tuple(alloc.tensor_shape), mybir.dt.np(alloc.dtype))
            )
    assert in_names == _IN_ORDER, in_names
    assert out_names == ["outT"], out_names
    in_names_full = tuple(in_names + out_names + ([partition_name] if partition_name else []))
    out_avals = tuple(out_avals)

    devices = jax.devices()[:NCORES]
    chains = [
        _build_chain(nc, devices[b * NKV : (b + 1) * NKV], in_names_full,
                     out_names, out_avals, partition_name)
        for b in range(B)
    ]
    return {"chains": chains, "nc": nc}


def _get_dispatch():
    if "chains" not in _STATE:
        _STATE.update(_build_dispatch())
    return _STATE


def _prep_weights(q_w, q_b, k_w, k_b, v_w, v_b, o_w, o_b):
    """Per-core weight slices, concatenated over the 4 head-groups (the same
    host arrays serve both batch meshes)."""
    qwT = np.ascontiguousarray(
        q_w.astype(BF).T.reshape(H, NKV, QD).transpose(1, 0, 2)
    ).reshape(NKV * H, QD)  # g-th [H,QD] block is q_w[g*QD:(g+1)*QD].T
    kwT = np.ascontiguousarray(
        k_w.astype(BF).T.reshape(H, NKV, HD).transpose(1, 0, 2)
    ).reshape(NKV * H, HD)
    vwT = np.ascontiguousarray(
        v_w.astype(BF).T.reshape(H, NKV, HD).transpose(1, 0, 2)
    ).reshape(NKV * H, HD)
    qb = np.ascontiguousarray(
        q_b.astype(np.float32).reshape(NKV, QH, P).transpose(0, 2, 1)
    ).reshape(NKV * P, QH)
    kb = k_b.astype(np.float32).reshape(NKV * P, 1)
    vb = v_b.astype(BF).reshape(NKV, HD)
    owT = np.ascontiguousarray(
        o_w.astype(BF).T.reshape(NKV, QH, P, H)
    ).reshape(NKV * QH, P, H)  # g-th block is o_w[:, g*QD:(g+1)*QD].T
    ob = o_b.astype(np.float32)  # (H,) -> local (512,) scatter chunk per core
    return {"qwT": qwT, "kwT": kwT, "vwT": vwT, "qb": qb, "kb": kb,
            "vb": vb, "owT": owT, "ob": ob}


def _weights_dev(q_w, q_b, k_w, k_b, v_w, v_b, o_w, o_b, chains):
    raw = (q_w, q_b, k_w, k_b, v_w, v_b, o_w, o_b)
    cached = _STATE.get("w_raw")
    if cached is not None and all(
        np.array_equal(a, b) for a, b in zip(cached, raw)
    ):
        return _STATE["w_dev"]
    host = _prep_weights(*raw)
    dev = []
    for ch in chains:
        d = {k: jax.device_put(v, ch["sharding"]) for k, v in host.items()}
        jax.block_until_ready(list(d.values()))
        dev.append(d)
    _STATE["w_raw"] = tuple(np.copy(a) for a in raw)
    _STATE["w_dev"] = dev
    return dev


_POOL = ThreadPoolExecutor(12)


def _upload_x(hidden_b, chain):
    """Per-shard pipelined upload: transpose strip g on host while strip g-1
    is already in flight (device_put is async)."""
    devs = chain["sharding"].mesh.devices.tolist()
    shards = [
        jax.device_put(
            np.ascontiguousarray(hidden_b[g * SG : (g + 1) * SG].T.astype(BF)),
            devs[g],
        )
        for g in range(NKV)
    ]
    return jax.make_array_from_single_device_arrays(
        (NKV * H, SG), chain["sharding"], shards
    )


def kernel(hidden_states, q_w, q_b, k_w, k_b, v_w, v_b, o_w, o_b):
    st = _get_dispatch()
    chains = st["chains"]
    hidden = np.asarray(hidden_states, dtype=np.float32)

    # dispatch batch 0's upload before doing any other host work
    x0g = chains[0]["gather"](_upload_x(hidden[0], chains[0]))
    wargs = [np.asarray(a, dtype=np.float32) for a in
             (q_w, q_b, k_w, k_b, v_w, v_b, o_w, o_b)]
    dev = _weights_dev(*wargs, chains)
    # Both chains dispatch from pool threads: warm calls keep the main
    # thread free to prep batch 1 while batch 0's dispatch proceeds, and on
    # a cold start the two chains' stage compiles (subprocess neuronx-cc,
    # GIL-released) run concurrently instead of serially.
    out = np.empty((B, S, H), np.float32)

    # Each batch's fetch jobs are submitted the moment its dispatch future
    # exists — before the other batch's host prep — so every D2H request is
    # in flight by the time the data becomes ready: tiny metas first, then
    # the q shards, with dequant of early shards overlapping later
    # transfers.
    def _finish_shard(fut, meta_fut, b, i):
        qsh = fut.result()[0].addressable_shards[i]
        p = qsh.index[0].start // R
        q = np.asarray(qsh.data).astype(np.float32)      # (R, S) uint8 -> f32
        meta = meta_fut.result()                         # (NKV*R, 2) fp32
        q *= meta[p * R : (p + 1) * R, 0:1]
        q += meta[p * R : (p + 1) * R, 1:2]
        out[b, :, p * R : (p + 1) * R] = q.T

    def _launch_fetches(fut, b):
        meta_fut = _POOL.submit(lambda: np.asarray(fut.result()[1]))
        return [
            _POOL.submit(_finish_shard, fut, meta_fut, b, i) for i in range(NKV)
        ]

    fut0 = _POOL.submit(chains[0]["rest"], x0g, dev[0])
    jobs = _launch_fetches(fut0, 0)
    x1 = _upload_x(hidden[1], chains[1])
    fut1 = _POOL.submit(chains[1]["fn"], x1, dev[1])
    jobs += _launch_fetches(fut1, 1)
    for j in jobs:
        j.result()
    return out
